# revision 9
# baseline (speedup 1.0000x reference)
"""MAMFGCN Trainium2 kernel: 6 snowball GCNs + attention fusion on 8 NeuronCores.

Strategy:
- Row-shard the node dim N=3000 across 8 cores (375 nodes each).
- The 6 snowballs (emb1,com1 | emb2,com2 | emb3,com3) are grouped into 3
  pairs, one per adjacency (sadj, fadj, fadj2), and run in lockstep.
- Everything on-chip lives feature-major ("transposed"): featsT tiles are
  [feat, node]. Heavy matmuls put the 375-node dim as the moving free dim
  (>=256 -> full PE rate).
- Per layer: XW^T = W^T @ featsT (per pair, both snowballs packed in 128
  PSUM partitions) -> PE-transpose to node-major -> ONE AllGather of the
  batched [3000, 384] XW (bf16) -> h^T = XWfull^T-stationary @ adjT-moving
  accumulated over 24 node k-tiles -> tanh(+bias) lands h^T directly in the
  next layer's featsT layout.
- Emission order: all of layer li+1's XW^T matmuls except the newest-h term
  are emitted right after AG(li) so the PE has work during the gather.
- DMAs are batched with 3D access patterns to keep the sync engine cheap.
- Adjacencies are host-transposed/padded once and stay SBUF-resident (bf16).
- Final attention/softmax head is computed on-chip in fp32.
"""
import os
import sys

sys.path.insert(0, "/opt/trn_rl_repo")
import numpy as np
import ml_dtypes

import concourse.bass as bass
import concourse.mybir as mybir
import concourse.tile as tile
from concourse import bacc
from concourse.bass_utils import run_bass_kernel_spmd
from concourse.masks import make_identity

dt = mybir.dt
AF = mybir.ActivationFunctionType
AX = mybir.AxisListType

N, NFEAT, NHID, NLAYERS, OUT, NCLASS = 3000, 256, 64, 9, 64, 2
NCORES = 8
NL = N // NCORES          # 375 local nodes
NPAD = 384                # per-rank padded rows for the AllGather
KN = NPAD * NCORES        # 3072 padded global nodes
KT = KN // 128            # 24 node k-tiles
KC = 4                    # xwf DMA chunks per layer (6 k-tiles each)
MB = 125                  # node-major m-chunk size (3 chunks of 125)
XWW = 6 * NHID            # 384 = width of the batched XW
bf16 = ml_dtypes.bfloat16

# tile-offset of each layer's weight tiles inside a pair's blob
_WOFF = []
_off = 0
for _i in range(NLAYERS):
    _WOFF.append(_off)
    _off += 2 + _i
_WOFF.append(_off)          # output layer: 2 + NLAYERS tiles
_WTILES = _off + 2 + NLAYERS  # 65 tiles per pair


def _pack_pair_weights(PA, PB):
    """[65*128, 128] f32 blob of PE-ready lhsT tiles for one pair."""
    tiles = []
    for i in range(NLAYERS + 1):
        WA = np.asarray(PA["Ws"][i] if i < NLAYERS else PA["Wo"], np.float32)
        WB = np.asarray(PB["Ws"][i] if i < NLAYERS else PB["Wo"], np.float32)
        nh = NLAYERS if i == NLAYERS else i
        for t in range(2):  # x-part: both snowballs packed on M
            tl = np.zeros((128, 128), np.float32)
            tl[:, 0:64] = WA[128 * t:128 * (t + 1), :]
            tl[:, 64:128] = WB[128 * t:128 * (t + 1), :]
            tiles.append(tl)
        for j in range(nh):  # h-part: block-diagonal (A top-left, B bottom-right)
            tl = np.zeros((128, 128), np.float32)
            tl[0:64, 0:64] = WA[256 + 64 * j:320 + 64 * j, :]
            tl[64:128, 64:128] = WB[256 + 64 * j:320 + 64 * j, :]
            tiles.append(tl)
    return np.concatenate(tiles, axis=0)


def _pack_pair_bias(PA, PB):
    """[128, 10] f32: col i = [b_A_i ; b_B_i], col 9 = output bias."""
    out = np.zeros((128, NLAYERS + 1), np.float32)
    for i in range(NLAYERS):
        out[0:64, i] = np.asarray(PA["bs"][i], np.float32)
        out[64:128, i] = np.asarray(PB["bs"][i], np.float32)
    out[0:64, NLAYERS] = np.asarray(PA["bo"], np.float32)
    out[64:128, NLAYERS] = np.asarray(PB["bo"], np.float32)
    return out


def _prep_adjT(adj, rows, dtype=bf16):
    """[KN, NL]: row 384*r+j = adj[rows, 375*r+j], pad rows zero."""
    out = np.zeros((KN, NL), dtype)
    a = np.asarray(adj, np.float32)
    for r in range(NCORES):
        blk = a[rows, NL * r:NL * (r + 1)].T.astype(dtype)  # [375, 375]
        out[NPAD * r:NPAD * r + NL, :] = blk
    return out


def build():
    nc = bacc.Bacc("TRN2", target_bir_lowering=False, debug=False,
                   num_devices=NCORES)

    xT_d = nc.dram_tensor("xT", [NFEAT, NL], dt.bfloat16, kind="ExternalInput")
    adjT_d = [nc.dram_tensor(f"adjT{p}", [KN, NL], dt.bfloat16,
                             kind="ExternalInput") for p in range(3)]
    wb_d = [nc.dram_tensor(f"wblob{p}", [_WTILES * 128, 128], dt.bfloat16,
                           kind="ExternalInput") for p in range(3)]
    bias_d = [nc.dram_tensor(f"bias{p}", [128, NLAYERS + 1], dt.float32,
                             kind="ExternalInput") for p in range(3)]
    attw1_d = nc.dram_tensor("attw1", [OUT, 2], dt.float32, kind="ExternalInput")
    attb1_d = nc.dram_tensor("attb1", [2, 1], dt.float32, kind="ExternalInput")
    attw2_d = nc.dram_tensor("attw2", [2, 1], dt.float32, kind="ExternalInput")
    mlpw_d = nc.dram_tensor("mlpw", [OUT, NCLASS], dt.float32, kind="ExternalInput")
    mlpb_d = nc.dram_tensor("mlpb", [NCLASS, 1], dt.float32, kind="ExternalInput")

    out_names = ["o_output", "o_beta", "o_emb1", "o_com1", "o_com2",
                 "o_com3", "o_emb2", "o_emb3"]
    o_output_d = nc.dram_tensor("o_output", [NL, NCLASS], dt.float32,
                                kind="ExternalOutput")
    o_beta_d = nc.dram_tensor("o_beta", [NL, 4], dt.float32,
                              kind="ExternalOutput")
    emb_d = {}
    for nm in out_names[2:]:
        emb_d[nm] = nc.dram_tensor(nm, [NL, OUT], dt.float32,
                                   kind="ExternalOutput")

    with tile.TileContext(nc) as tc:
        with (
            tc.tile_pool(name="const", bufs=1) as pc,
            tc.tile_pool(name="wts", bufs=4) as pw,
            tc.tile_pool(name="xwf", bufs=6) as pxwf,
            tc.tile_pool(name="rot", bufs=2) as pr,
            tc.tile_pool(name="dram", bufs=1, space="DRAM") as pd,
        ):
            # ---- persistent/const tiles ----
            ident_b = pc.tile([128, 128], dt.bfloat16, name="ident_b")
            make_identity(nc, ident_b[:])
            ident_f = pc.tile([128, 128], dt.float32, name="ident_f")
            make_identity(nc, ident_f[:])
            zeros_b = pc.tile([128, XWW], dt.bfloat16, name="zeros_b")
            nc.gpsimd.memset(zeros_b[:], 0.0)

            xT = []
            for t in range(2):
                xt = pc.tile([128, NL], dt.bfloat16, name=f"xT_{t}")
                nc.sync.dma_start(xt[:], xT_d[128 * t:128 * (t + 1), :])
                xT.append(xt)

            bias = []
            for p in range(3):
                b = pc.tile([128, NLAYERS + 1], dt.float32, name=f"bias_{p}")
                nc.sync.dma_start(b[:], bias_d[p][:])
                bias.append(b)

            # adjacencies: SBUF-resident as 6 chunk-tiles of 4 k-tiles per
            # matrix, loaded round-robin so every pair's early k-tiles land
            # early and the DMAs spread across queues.
            ACH = 4
            adjC = [[None] * (KT // ACH) for _ in range(3)]
            for c in range(KT // ACH):
                for p in range(3):
                    a = pc.tile([128, ACH, NL], dt.bfloat16,
                                name=f"adjT_{p}_{c}")
                    nc.sync.dma_start(
                        a[:], adjT_d[p][128 * ACH * c:128 * ACH * (c + 1), :]
                        .rearrange("(t p) n -> p t n", p=128))
                    adjC[p][c] = a

            attw1 = pc.tile([OUT, 2], dt.float32, name="attw1")
            nc.sync.dma_start(attw1[:], attw1_d[:])
            attb1 = pc.tile([2, 1], dt.float32, name="attb1")
            nc.sync.dma_start(attb1[:], attb1_d[:])
            attw2 = pc.tile([2, 1], dt.float32, name="attw2")
            nc.sync.dma_start(attw2[:], attw2_d[:])
            mlpw = pc.tile([OUT, NCLASS], dt.float32, name="mlpw")
            nc.sync.dma_start(mlpw[:], mlpw_d[:])
            mlpb = pc.tile([NCLASS, 1], dt.float32, name="mlpb")
            nc.sync.dma_start(mlpb[:], mlpb_d[:])

            eps_t = pc.tile([128, 1], dt.float32, name="eps_t")
            nc.gpsimd.memset(eps_t[:], 1e-24)
            third_t = pc.tile([128, 1], dt.float32, name="third_t")
            nc.gpsimd.memset(third_t[:], 1.0 / 3.0)

            hT = [[None] * NLAYERS for _ in range(3)]  # persistent h tiles
            oT = [None] * 3

            # ---- layer loop ----
            ps_main = tc.tile_pool(name="ps_main", bufs=2, space="PSUM")
            ps = ps_main.__enter__()

            def load_w(li, p, t0, ntl, nm):
                """One DMA for weight tiles t0..t0+ntl of (layer, pair)."""
                w = pw.tile([128, ntl, 128], dt.bfloat16, tag="wt",
                            padded_shape=[128, NLAYERS + 2, 128], bufs=4,
                            name=nm)
                r0 = (_WOFF[li] + t0) * 128
                nc.sync.dma_start(
                    w[:], wb_d[p][r0:r0 + ntl * 128, :]
                    .rearrange("(t p) m -> p t m", p=128))
                return w

            # prologue: layer-0 "partials" (layer 0 has no h terms)
            pxw_cur = []
            for p in range(3):
                pxw = ps.tile([128, NL], dt.float32, tag="pxw", bufs=3,
                              name=f"pxw_0_{p}")
                w = load_w(0, p, 0, 2, f"w_0_{p}")
                nc.tensor.matmul(pxw[:], w[:, 0, :], xT[0][:], start=True,
                                 stop=False)
                nc.tensor.matmul(pxw[:], w[:, 1, :], xT[1][:], start=False,
                                 stop=True)
                pxw_cur.append(pxw)

            for li in range(NLAYERS + 1):
                is_out = li == NLAYERS

                cc_in = pd.tile([NPAD, XWW], dt.bfloat16, name=f"cc_in_{li}")
                cc_out = pd.tile([KN, XWW], dt.bfloat16, name=f"cc_out_{li}",
                                 addr_space="Shared")
                nc.sync.dma_start(cc_in[NL:NPAD, :], zeros_b[0:NPAD - NL, :])

                # finish XW^T(li): add the newest-h term, transpose, stage
                ccs = [pr.tile([MB, XWW], dt.bfloat16, tag="ccs", bufs=6,
                               name=f"ccs_{li}_{m}") for m in range(3)]
                for p in range(3):
                    pxw = pxw_cur[p]
                    if li >= 1:
                        wlast = load_w(li, p, 2 + (li - 1), 1, f"wl_{li}_{p}")
                        nc.tensor.matmul(pxw[:], wlast[:, 0, :],
                                         hT[p][li - 1][:],
                                         start=False, stop=True)
                    xwtT = pr.tile([128, NL], dt.bfloat16, tag="xwtT", bufs=3,
                                   name=f"xwtT_{li}_{p}")
                    nc.scalar.activation(xwtT[:], pxw[:], AF.Copy)
                    for m in range(3):
                        ptr = ps.tile([MB, 128], dt.bfloat16, tag="ptr",
                                      bufs=2, name=f"ptr_{li}_{p}_{m}")
                        nc.tensor.transpose(
                            ptr[:], xwtT[:, MB * m:MB * (m + 1)], ident_b[:])
                        nc.vector.tensor_copy(
                            ccs[m][:, 128 * p:128 * (p + 1)], ptr[:])
                for m in range(3):
                    nc.sync.dma_start(cc_in[MB * m:MB * (m + 1), :], ccs[m][:])

                nc.gpsimd.collective_compute(
                    "AllGather", mybir.AluOpType.bypass,
                    replica_groups=[list(range(NCORES))],
                    ins=[cc_in.opt()], outs=[cc_out.opt()],
                )

                # partials for XW^T(li+1): x-part + h_0..h_{li-1} terms,
                # runnable while the AllGather is in flight.
                if not is_out:
                    pxw_next = []
                    for p in range(3):
                        pxw = ps.tile([128, NL], dt.float32, tag="pxw",
                                      bufs=3, name=f"pxw_{li + 1}_{p}")
                        w = load_w(li + 1, p, 0, 2 + li, f"w_{li + 1}_{p}")
                        nc.tensor.matmul(pxw[:], w[:, 0, :], xT[0][:],
                                         start=True, stop=False)
                        nc.tensor.matmul(pxw[:], w[:, 1, :], xT[1][:],
                                         start=False, stop=False)
                        for j in range(li):
                            nc.tensor.matmul(pxw[:], w[:, 2 + j, :],
                                             hT[p][j][:],
                                             start=False, stop=False)
                        pxw_next.append(pxw)
                    pxw_cur = pxw_next

                # adj matmuls for layer li; xwf arrives in 4 chunked DMAs
                xwf = []
                for kc in range(KC):
                    xf = pxwf.tile([128, KT // KC, XWW], dt.bfloat16,
                                   tag="xwf", name=f"xwf_{li}_{kc}")
                    r0 = kc * (KN // KC)
                    nc.sync.dma_start(
                        xf[:], cc_out[r0:r0 + KN // KC, :]
                        .rearrange("(t p) f -> p t f", p=128))
                    xwf.append(xf)

                for p in range(3):
                    ph = ps.tile([128, NL], dt.float32, tag="ph", bufs=3,
                                 name=f"ph_{li}_{p}")
                    for kt in range(KT):
                        nc.tensor.matmul(
                            ph[:],
                            xwf[kt // 6][:, kt % 6, 128 * p:128 * (p + 1)],
                            adjC[p][kt // ACH][:, kt % ACH, :],
                            start=(kt == 0), stop=(kt == KT - 1))
                    if not is_out:
                        h = pc.tile([128, NL], dt.bfloat16,
                                    name=f"hT_{p}_{li}")
                        nc.scalar.activation(h[:], ph[:], AF.Tanh,
                                             bias=bias[p][:, li:li + 1])
                        hT[p][li] = h
                    else:
                        o = pc.tile([128, NL], dt.float32, name=f"oT_{p}")
                        nc.vector.tensor_scalar_add(o[:], ph[:],
                                                    bias[p][:, li:li + 1])
                        oT[p] = o

            ps_main.__exit__(None, None, None)
            ps_head = tc.tile_pool(name="ps_head", bufs=8, space="PSUM")
            ps = ps_head.__enter__()

            # ---- head (fp32) ----
            # snowball order inside pair tiles: cols [0:64]=A, [64:128]=B
            # block order across [125, 384] node-major tiles:
            #   0=emb1 1=com1 2=emb2 3=com2 4=emb3 5=com3
            blk_out = ["o_emb1", "o_com1", "o_emb2", "o_com2", "o_emb3",
                       "o_com3"]
            zT = [pc.tile([64, NL], dt.float32, name=f"zT_{k}")
                  for k in range(4)]
            z_nm_store = []
            for m in range(3):
                sl = slice(MB * m, MB * (m + 1))
                o_nm = pr.tile([MB, 384], dt.float32, tag="onm", bufs=3,
                               name=f"onm_{m}")
                for p in range(3):
                    pt = ps.tile([MB, 128], dt.float32, tag="hps",
                                 name=f"pho_{m}_{p}")
                    nc.tensor.transpose(pt[:], oT[p][:, sl], ident_f[:])
                    nc.vector.tensor_copy(o_nm[:, 128 * p:128 * (p + 1)],
                                          pt[:])

                # batched row-normalize of all 6 blocks
                sq = pr.tile([MB, 384], dt.float32, tag="sq", bufs=3,
                             name=f"sq_{m}")
                nc.vector.tensor_mul(sq[:], o_nm[:], o_nm[:])
                nrm6 = pr.tile([MB, 6], dt.float32, tag="nrm6", bufs=3,
                               name=f"nrm6_{m}")
                nc.vector.reduce_sum(nrm6[:],
                                     sq[:].rearrange("p (s f) -> p s f", f=64),
                                     axis=AX.X)
                nc.vector.tensor_scalar_max(nrm6[:], nrm6[:], eps_t[0:MB, :])
                nc.scalar.activation(nrm6[:], nrm6[:], AF.Sqrt)
                nc.vector.reciprocal(nrm6[:], nrm6[:])

                en = {}
                for k, nm in enumerate(blk_out):
                    e = pc.tile([MB, 64], dt.float32, name=f"en_{m}_{nm}")
                    nc.vector.tensor_scalar_mul(
                        e[:], o_nm[:, 64 * k:64 * (k + 1)], nrm6[:, k:k + 1])
                    nc.sync.dma_start(emb_d[nm][sl, :], e[:])
                    en[nm] = e

                xcom = pc.tile([MB, 64], dt.float32, name=f"xcom_{m}")
                nc.vector.tensor_add(xcom[:], en["o_com1"][:], en["o_com2"][:])
                nc.vector.tensor_add(xcom[:], xcom[:], en["o_com3"][:])
                nc.vector.tensor_scalar_mul(xcom[:], xcom[:], third_t[0:MB, :])

                z_nm = {0: en["o_emb1"], 1: en["o_emb2"], 2: en["o_emb3"],
                        3: xcom}
                z_nm_store.append(z_nm)
                for k in range(4):
                    ptz = ps.tile([64, MB], dt.float32, tag="hps",
                                  name=f"ptz_{m}_{k}")
                    nc.tensor.transpose(ptz[:], z_nm[k][:],
                                        ident_f[0:MB, 0:MB])
                    nc.vector.tensor_copy(zT[k][:, sl], ptz[:])

            # attention scores w_k: [1, 375] each
            wsb = []
            for k in range(4):
                pa = ps.tile([2, NL], dt.float32, tag="hps", name=f"pa_{k}")
                nc.tensor.matmul(pa[:], attw1[:], zT[k][:], start=True,
                                 stop=True)
                a1 = pr.tile([2, NL], dt.float32, tag="a1", bufs=4,
                             name=f"a1_{k}")
                nc.scalar.activation(a1[:], pa[:], AF.Tanh, bias=attb1[:])
                pk = ps.tile([1, NL], dt.float32, tag="hps", name=f"pk_{k}")
                nc.tensor.matmul(pk[:], attw2[:], a1[:], start=True, stop=True)
                w = pc.tile([1, NL], dt.float32, name=f"wsb_{k}")
                nc.vector.tensor_copy(w[:], pk[:])
                wsb.append(w)

            # softmax over the 4 scores (all partition-0 rows)
            mx = pc.tile([1, NL], dt.float32, name="mx")
            nc.vector.tensor_max(mx[:], wsb[0][:], wsb[1][:])
            nc.vector.tensor_max(mx[:], mx[:], wsb[2][:])
            nc.vector.tensor_max(mx[:], mx[:], wsb[3][:])
            es = []
            for k in range(4):
                e = pc.tile([1, NL], dt.float32, name=f"es_{k}")
                nc.vector.tensor_sub(e[:], wsb[k][:], mx[:])
                nc.scalar.activation(e[:], e[:], AF.Exp)
                es.append(e)
            ssum4 = pc.tile([1, NL], dt.float32, name="ssum4")
            nc.vector.tensor_add(ssum4[:], es[0][:], es[1][:])
            nc.vector.tensor_add(ssum4[:], ssum4[:], es[2][:])
            nc.vector.tensor_add(ssum4[:], ssum4[:], es[3][:])
            rcp4 = pc.tile([1, NL], dt.float32, name="rcp4")
            nc.vector.reciprocal(rcp4[:], ssum4[:])
            betas = []
            for k in range(4):
                b = pc.tile([1, NL], dt.float32, name=f"beta_{k}")
                nc.vector.tensor_mul(b[:], es[k][:], rcp4[:])
                betas.append(b)

            # beta -> node-major + emb = sum beta_k * z_k, logits, softmax
            embT = pc.tile([64, NL], dt.float32, name="embT")
            for m in range(3):
                sl = slice(MB * m, MB * (m + 1))
                beta_nm = pr.tile([MB, 4], dt.float32, tag="betanm", bufs=3,
                                  name=f"betanm_{m}")
                for k in range(4):
                    ptb = ps.tile([MB, 1], dt.float32, tag="hps",
                                  name=f"ptb_{m}_{k}")
                    nc.tensor.transpose(ptb[:], betas[k][:, sl],
                                        ident_f[0:1, 0:1])
                    nc.vector.tensor_copy(beta_nm[:, k:k + 1], ptb[:])
                nc.sync.dma_start(o_beta_d[sl, :], beta_nm[:])

                z_nm = z_nm_store[m]
                emb_nm = pr.tile([MB, 64], dt.float32, tag="embnm", bufs=2,
                                 name=f"embnm_{m}")
                tmp = pr.tile([MB, 64], dt.float32, tag="tmpnm", bufs=2,
                              name=f"tmpnm_{m}")
                nc.vector.tensor_scalar_mul(emb_nm[:], z_nm[0][:],
                                            beta_nm[:, 0:1])
                for k in range(1, 4):
                    nc.vector.tensor_scalar_mul(tmp[:], z_nm[k][:],
                                                beta_nm[:, k:k + 1])
                    nc.vector.tensor_add(emb_nm[:], emb_nm[:], tmp[:])

                pte = ps.tile([64, MB], dt.float32, tag="hps",
                              name=f"pte_{m}")
                nc.tensor.transpose(pte[:], emb_nm[:], ident_f[0:MB, 0:MB])
                nc.vector.tensor_copy(embT[:, sl], pte[:])

            pl = ps.tile([NCLASS, NL], dt.float32, tag="hps", name="pl")
            nc.tensor.matmul(pl[:], mlpw[:], embT[:], start=True, stop=True)
            lg = pc.tile([NCLASS, NL], dt.float32, name="lg")
            nc.vector.tensor_scalar_add(lg[:], pl[:], mlpb[:])
            for m in range(3):
                sl = slice(MB * m, MB * (m + 1))
                ptl = ps.tile([MB, NCLASS], dt.float32, tag="hps",
                              name=f"ptl_{m}")
                nc.tensor.transpose(ptl[:], lg[:, sl], ident_f[0:2, 0:2])
                lgn = pr.tile([MB, NCLASS], dt.float32, tag="lgn", bufs=2,
                              name=f"lgn_{m}")
                nc.vector.tensor_copy(lgn[:], ptl[:])
                lmx = pr.tile([MB, 1], dt.float32, tag="lmx", bufs=2,
                              name=f"lmx_{m}")
                nc.vector.reduce_max(lmx[:], lgn[:], axis=AX.X)
                nc.vector.tensor_scalar_sub(lgn[:], lgn[:], lmx[:])
                nc.scalar.activation(lgn[:], lgn[:], AF.Exp)
                lsm = pr.tile([MB, 1], dt.float32, tag="lsm", bufs=2,
                              name=f"lsm_{m}")
                nc.vector.reduce_sum(lsm[:], lgn[:], axis=AX.X)
                lrc = pr.tile([MB, 1], dt.float32, tag="lrc", bufs=2,
                              name=f"lrc_{m}")
                nc.vector.reciprocal(lrc[:], lsm[:])
                nc.vector.tensor_scalar_mul(lgn[:], lgn[:], lrc[:])
                nc.sync.dma_start(o_output_d[sl, :], lgn[:])
            ps_head.__exit__(None, None, None)

    nc.compile()
    return nc


_NC_CACHE = None


def _get_nc():
    global _NC_CACHE
    if _NC_CACHE is None:
        _NC_CACHE = build()
    return _NC_CACHE


def kernel(x, sadj, fadj, fadj2, sgcn1, sgcn2, sgcn3, cgcn,
           att_w1, att_b1, att_w2, mlp_w, mlp_b):
    x = np.asarray(x, np.float32)
    pairs = [(sgcn1, cgcn), (sgcn2, cgcn), (sgcn3, cgcn)]
    adjs = [sadj, fadj, fadj2]

    wblobs = [_pack_pair_weights(PA, PB).astype(bf16) for PA, PB in pairs]
    biases = [_pack_pair_bias(PA, PB) for PA, PB in pairs]
    shared = {
        "attw1": np.asarray(att_w1, np.float32).reshape(OUT, 2),
        "attb1": np.asarray(att_b1, np.float32).reshape(2, 1),
        "attw2": np.asarray(att_w2, np.float32).reshape(2, 1),
        "mlpw": np.asarray(mlp_w, np.float32).reshape(OUT, NCLASS),
        "mlpb": np.asarray(mlp_b, np.float32).reshape(NCLASS, 1),
    }

    in_maps = []
    for c in range(NCORES):
        rows = slice(NL * c, NL * (c + 1))
        m = {
            "xT": np.ascontiguousarray(x[rows].T).astype(bf16),
        }
        for p in range(3):
            m[f"adjT{p}"] = _prep_adjT(adjs[p], rows)
            m[f"wblob{p}"] = wblobs[p]
            m[f"bias{p}"] = biases[p]
        m.update(shared)
        in_maps.append(m)

    nc = _get_nc()
    trace = bool(int(os.environ.get("KERNEL_TRACE", "0")))
    res = run_bass_kernel_spmd(nc, in_maps, core_ids=list(range(NCORES)),
                               trace=trace)
    if trace:
        kernel.last_exec_time_ns = res.exec_time_ns
        kernel.last_results = res

    def cat(name):
        return np.concatenate([res.results[c][name] for c in range(NCORES)],
                              axis=0)

    output = cat("o_output")
    beta = cat("o_beta").reshape(N, 4, 1)
    emb1 = cat("o_emb1")
    com1 = cat("o_com1")
    com2 = cat("o_com2")
    com3 = cat("o_com3")
    emb2 = cat("o_emb2")
    emb3 = cat("o_emb3")
    return (output, beta, emb1, com1, com2, com3, emb2, emb3)


# revision 10
# speedup vs baseline: 1.0039x; 1.0039x over previous
"""MAMFGCN Trainium2 kernel: 6 snowball GCNs + attention fusion on 8 NeuronCores.

Strategy:
- Row-shard the node dim N=3000 across 8 cores (375 nodes each).
- The 6 snowballs (emb1,com1 | emb2,com2 | emb3,com3) are grouped into 3
  pairs, one per adjacency (sadj, fadj, fadj2), and run in lockstep.
- Everything on-chip lives feature-major ("transposed"): featsT tiles are
  [feat, node]. Heavy matmuls put the 375-node dim as the moving free dim
  (>=256 -> full PE rate).
- Per layer: XW^T = W^T @ featsT (per pair, both snowballs packed in 128
  PSUM partitions) -> PE-transpose to node-major -> ONE AllGather of the
  batched [3000, 384] XW (bf16) -> h^T = XWfull^T-stationary @ adjT-moving
  accumulated over 24 node k-tiles -> tanh(+bias) lands h^T directly in the
  next layer's featsT layout.
- Emission order: all of layer li+1's XW^T matmuls except the newest-h term
  are emitted right after AG(li) so the PE has work during the gather.
- DMAs are batched with 3D access patterns to keep the sync engine cheap.
- Adjacencies are host-transposed/padded once and stay SBUF-resident (bf16).
- Final attention/softmax head is computed on-chip in fp32.
"""
import os
import sys

sys.path.insert(0, "/opt/trn_rl_repo")
import numpy as np
import ml_dtypes

import concourse.bass as bass
import concourse.mybir as mybir
import concourse.tile as tile
from concourse import bacc
from concourse.bass_utils import run_bass_kernel_spmd
from concourse.masks import make_identity

dt = mybir.dt
AF = mybir.ActivationFunctionType
AX = mybir.AxisListType

N, NFEAT, NHID, NLAYERS, OUT, NCLASS = 3000, 256, 64, 9, 64, 2
NCORES = 8
NL = N // NCORES          # 375 local nodes
NPAD = 384                # per-rank padded rows for the AllGather
KN = NPAD * NCORES        # 3072 padded global nodes
KT = KN // 128            # 24 node k-tiles
KC = 4                    # xwf DMA chunks per layer (6 k-tiles each)
MB = 125                  # node-major m-chunk size (3 chunks of 125)
XWW = 6 * NHID            # 384 = width of the batched XW
bf16 = ml_dtypes.bfloat16

# tile-offset of each layer's weight tiles inside a pair's blob
_WOFF = []
_off = 0
for _i in range(NLAYERS):
    _WOFF.append(_off)
    _off += 2 + _i
_WOFF.append(_off)          # output layer: 2 + NLAYERS tiles
_WTILES = _off + 2 + NLAYERS  # 65 tiles per pair


def _pack_pair_weights(PA, PB):
    """[65*128, 128] f32 blob of PE-ready lhsT tiles for one pair."""
    tiles = []
    for i in range(NLAYERS + 1):
        WA = np.asarray(PA["Ws"][i] if i < NLAYERS else PA["Wo"], np.float32)
        WB = np.asarray(PB["Ws"][i] if i < NLAYERS else PB["Wo"], np.float32)
        nh = NLAYERS if i == NLAYERS else i
        for t in range(2):  # x-part: both snowballs packed on M
            tl = np.zeros((128, 128), np.float32)
            tl[:, 0:64] = WA[128 * t:128 * (t + 1), :]
            tl[:, 64:128] = WB[128 * t:128 * (t + 1), :]
            tiles.append(tl)
        for j in range(nh):  # h-part: block-diagonal (A top-left, B bottom-right)
            tl = np.zeros((128, 128), np.float32)
            tl[0:64, 0:64] = WA[256 + 64 * j:320 + 64 * j, :]
            tl[64:128, 64:128] = WB[256 + 64 * j:320 + 64 * j, :]
            tiles.append(tl)
    return np.concatenate(tiles, axis=0)


def _pack_pair_bias(PA, PB):
    """[128, 10] f32: col i = [b_A_i ; b_B_i], col 9 = output bias."""
    out = np.zeros((128, NLAYERS + 1), np.float32)
    for i in range(NLAYERS):
        out[0:64, i] = np.asarray(PA["bs"][i], np.float32)
        out[64:128, i] = np.asarray(PB["bs"][i], np.float32)
    out[0:64, NLAYERS] = np.asarray(PA["bo"], np.float32)
    out[64:128, NLAYERS] = np.asarray(PB["bo"], np.float32)
    return out


def _prep_adjT_img(adj, rows):
    """SBUF image [128, KT*NL]: partition p, col kt*NL+n = adjT[128*kt+p, n]."""
    a = _prep_adjT(adj, rows)                       # [KN, NL]
    return np.ascontiguousarray(
        a.reshape(KT, 128, NL).transpose(1, 0, 2).reshape(128, KT * NL))


def _pack_weights_img(PA, PB):
    """SBUF image [128, 65*128] of the pair's lhsT tiles."""
    b = _pack_pair_weights(PA, PB).astype(bf16)     # [65*128, 128]
    return np.ascontiguousarray(
        b.reshape(_WTILES, 128, 128).transpose(1, 0, 2).reshape(128, _WTILES * 128))


def _prep_adjT(adj, rows, dtype=bf16):
    """[KN, NL]: row 384*r+j = adj[rows, 375*r+j], pad rows zero."""
    out = np.zeros((KN, NL), dtype)
    a = np.asarray(adj, np.float32)
    for r in range(NCORES):
        blk = a[rows, NL * r:NL * (r + 1)].T.astype(dtype)  # [375, 375]
        out[NPAD * r:NPAD * r + NL, :] = blk
    return out


def build():
    nc = bacc.Bacc("TRN2", target_bir_lowering=False, debug=False,
                   num_devices=NCORES)

    xT_d = nc.dram_tensor("xT", [NFEAT, NL], dt.bfloat16, kind="ExternalInput")
    adjT_d = [nc.dram_tensor(f"adjT{p}", [128, KT * NL], dt.bfloat16,
                             kind="ExternalInput") for p in range(3)]
    wb_d = [nc.dram_tensor(f"wblob{p}", [128, _WTILES * 128], dt.bfloat16,
                           kind="ExternalInput") for p in range(3)]
    bias_d = [nc.dram_tensor(f"bias{p}", [128, NLAYERS + 1], dt.float32,
                             kind="ExternalInput") for p in range(3)]
    attw1_d = nc.dram_tensor("attw1", [OUT, 2], dt.float32, kind="ExternalInput")
    attb1_d = nc.dram_tensor("attb1", [2, 1], dt.float32, kind="ExternalInput")
    attw2_d = nc.dram_tensor("attw2", [2, 1], dt.float32, kind="ExternalInput")
    mlpw_d = nc.dram_tensor("mlpw", [OUT, NCLASS], dt.float32, kind="ExternalInput")
    mlpb_d = nc.dram_tensor("mlpb", [NCLASS, 1], dt.float32, kind="ExternalInput")

    out_names = ["o_output", "o_beta", "o_emb1", "o_com1", "o_com2",
                 "o_com3", "o_emb2", "o_emb3"]
    o_output_d = nc.dram_tensor("o_output", [NL, NCLASS], dt.float32,
                                kind="ExternalOutput")
    o_beta_d = nc.dram_tensor("o_beta", [NL, 4], dt.float32,
                              kind="ExternalOutput")
    emb_d = {}
    for nm in out_names[2:]:
        emb_d[nm] = nc.dram_tensor(nm, [NL, OUT], dt.float32,
                                   kind="ExternalOutput")

    with tile.TileContext(nc) as tc:
        with (
            tc.tile_pool(name="const", bufs=1) as pc,
            tc.tile_pool(name="wts", bufs=4) as pw,
            tc.tile_pool(name="xwf", bufs=6) as pxwf,
            tc.tile_pool(name="rot", bufs=2) as pr,
            tc.tile_pool(name="dram", bufs=1, space="DRAM") as pd,
        ):
            # ---- persistent/const tiles ----
            ident_b = pc.tile([128, 128], dt.bfloat16, name="ident_b")
            make_identity(nc, ident_b[:])
            ident_f = pc.tile([128, 128], dt.float32, name="ident_f")
            make_identity(nc, ident_f[:])
            zeros_b = pc.tile([128, XWW], dt.bfloat16, name="zeros_b")
            nc.gpsimd.memset(zeros_b[:], 0.0)

            xT = []
            for t in range(2):
                xt = pc.tile([128, NL], dt.bfloat16, name=f"xT_{t}")
                nc.sync.dma_start(xt[:], xT_d[128 * t:128 * (t + 1), :])
                xT.append(xt)

            bias = []
            for p in range(3):
                b = pc.tile([128, NLAYERS + 1], dt.float32, name=f"bias_{p}")
                nc.sync.dma_start(b[:], bias_d[p][:])
                bias.append(b)

            # adjacencies: SBUF image layout, 2 contiguous DMAs per matrix
            adjT = []
            for p in range(3):
                a = pc.tile([128, KT * NL], dt.bfloat16, name=f"adjT_{p}")
                half = KT * NL // 2
                nc.sync.dma_start(a[:, 0:half], adjT_d[p][:, 0:half])
                nc.sync.dma_start(a[:, half:], adjT_d[p][:, half:])
                adjT.append(a)

            attw1 = pc.tile([OUT, 2], dt.float32, name="attw1")
            nc.sync.dma_start(attw1[:], attw1_d[:])
            attb1 = pc.tile([2, 1], dt.float32, name="attb1")
            nc.sync.dma_start(attb1[:], attb1_d[:])
            attw2 = pc.tile([2, 1], dt.float32, name="attw2")
            nc.sync.dma_start(attw2[:], attw2_d[:])
            mlpw = pc.tile([OUT, NCLASS], dt.float32, name="mlpw")
            nc.sync.dma_start(mlpw[:], mlpw_d[:])
            mlpb = pc.tile([NCLASS, 1], dt.float32, name="mlpb")
            nc.sync.dma_start(mlpb[:], mlpb_d[:])

            eps_t = pc.tile([128, 1], dt.float32, name="eps_t")
            nc.gpsimd.memset(eps_t[:], 1e-24)
            third_t = pc.tile([128, 1], dt.float32, name="third_t")
            nc.gpsimd.memset(third_t[:], 1.0 / 3.0)

            hT = [[None] * NLAYERS for _ in range(3)]  # persistent h tiles
            oT = [None] * 3

            # ---- layer loop ----
            ps_main = tc.tile_pool(name="ps_main", bufs=2, space="PSUM")
            ps = ps_main.__enter__()

            def load_w(li, p, t0, ntl, nm):
                """One contiguous DMA for weight tiles t0..t0+ntl."""
                w = pw.tile([128, ntl, 128], dt.bfloat16, tag="wt",
                            padded_shape=[128, NLAYERS + 2, 128], bufs=4,
                            name=nm)
                c0 = (_WOFF[li] + t0) * 128
                nc.sync.dma_start(
                    w[:], wb_d[p][:, c0:c0 + ntl * 128]
                    .rearrange("p (t m) -> p t m", m=128))
                return w

            # prologue: layer-0 "partials" (layer 0 has no h terms)
            pxw_cur = []
            for p in range(3):
                pxw = ps.tile([128, NL], dt.float32, tag="pxw", bufs=3,
                              name=f"pxw_0_{p}")
                w = load_w(0, p, 0, 2, f"w_0_{p}")
                nc.tensor.matmul(pxw[:], w[:, 0, :], xT[0][:], start=True,
                                 stop=False)
                nc.tensor.matmul(pxw[:], w[:, 1, :], xT[1][:], start=False,
                                 stop=True)
                pxw_cur.append(pxw)

            for li in range(NLAYERS + 1):
                is_out = li == NLAYERS

                cc_in = pd.tile([NPAD, XWW], dt.bfloat16, name=f"cc_in_{li}")
                cc_out = pd.tile([KN, XWW], dt.bfloat16, name=f"cc_out_{li}",
                                 addr_space="Shared")
                nc.sync.dma_start(cc_in[NL:NPAD, :], zeros_b[0:NPAD - NL, :])

                # finish XW^T(li): add the newest-h term, transpose, stage
                ccs = [pr.tile([MB, XWW], dt.bfloat16, tag="ccs", bufs=6,
                               name=f"ccs_{li}_{m}") for m in range(3)]
                for p in range(3):
                    pxw = pxw_cur[p]
                    if li >= 1:
                        wlast = load_w(li, p, 2 + (li - 1), 1, f"wl_{li}_{p}")
                        nc.tensor.matmul(pxw[:], wlast[:, 0, :],
                                         hT[p][li - 1][:],
                                         start=False, stop=True)
                    xwtT = pr.tile([128, NL], dt.bfloat16, tag="xwtT", bufs=3,
                                   name=f"xwtT_{li}_{p}")
                    nc.scalar.activation(xwtT[:], pxw[:], AF.Copy)
                    for m in range(3):
                        ptr = ps.tile([MB, 128], dt.bfloat16, tag="ptr",
                                      bufs=2, name=f"ptr_{li}_{p}_{m}")
                        nc.tensor.transpose(
                            ptr[:], xwtT[:, MB * m:MB * (m + 1)], ident_b[:])
                        nc.vector.tensor_copy(
                            ccs[m][:, 128 * p:128 * (p + 1)], ptr[:])
                for m in range(3):
                    nc.sync.dma_start(cc_in[MB * m:MB * (m + 1), :], ccs[m][:])

                nc.gpsimd.collective_compute(
                    "AllGather", mybir.AluOpType.bypass,
                    replica_groups=[list(range(NCORES))],
                    ins=[cc_in.opt()], outs=[cc_out.opt()],
                )

                # partials for XW^T(li+1): x-part + h_0..h_{li-1} terms,
                # runnable while the AllGather is in flight.
                if not is_out:
                    pxw_next = []
                    for p in range(3):
                        pxw = ps.tile([128, NL], dt.float32, tag="pxw",
                                      bufs=3, name=f"pxw_{li + 1}_{p}")
                        w = load_w(li + 1, p, 0, 2 + li, f"w_{li + 1}_{p}")
                        nc.tensor.matmul(pxw[:], w[:, 0, :], xT[0][:],
                                         start=True, stop=False)
                        nc.tensor.matmul(pxw[:], w[:, 1, :], xT[1][:],
                                         start=False, stop=False)
                        for j in range(li):
                            nc.tensor.matmul(pxw[:], w[:, 2 + j, :],
                                             hT[p][j][:],
                                             start=False, stop=False)
                        pxw_next.append(pxw)
                    pxw_cur = pxw_next

                # adj matmuls for layer li; xwf arrives in 4 chunked DMAs
                xwf = []
                for kc in range(KC):
                    xf = pxwf.tile([128, KT // KC, XWW], dt.bfloat16,
                                   tag="xwf", name=f"xwf_{li}_{kc}")
                    r0 = kc * (KN // KC)
                    nc.sync.dma_start(
                        xf[:], cc_out[r0:r0 + KN // KC, :]
                        .rearrange("(t p) f -> p t f", p=128))
                    xwf.append(xf)

                for p in range(3):
                    ph = ps.tile([128, NL], dt.float32, tag="ph", bufs=3,
                                 name=f"ph_{li}_{p}")
                    for kt in range(KT):
                        nc.tensor.matmul(
                            ph[:],
                            xwf[kt // 6][:, kt % 6, 128 * p:128 * (p + 1)],
                            adjT[p][:, kt * NL:(kt + 1) * NL],
                            start=(kt == 0), stop=(kt == KT - 1))
                    if not is_out:
                        h = pc.tile([128, NL], dt.bfloat16,
                                    name=f"hT_{p}_{li}")
                        nc.scalar.activation(h[:], ph[:], AF.Tanh,
                                             bias=bias[p][:, li:li + 1])
                        hT[p][li] = h
                    else:
                        o = pc.tile([128, NL], dt.float32, name=f"oT_{p}")
                        nc.vector.tensor_scalar_add(o[:], ph[:],
                                                    bias[p][:, li:li + 1])
                        oT[p] = o

            ps_main.__exit__(None, None, None)
            ps_head = tc.tile_pool(name="ps_head", bufs=8, space="PSUM")
            ps = ps_head.__enter__()

            # ---- head (fp32) ----
            # snowball order inside pair tiles: cols [0:64]=A, [64:128]=B
            # block order across [125, 384] node-major tiles:
            #   0=emb1 1=com1 2=emb2 3=com2 4=emb3 5=com3
            blk_out = ["o_emb1", "o_com1", "o_emb2", "o_com2", "o_emb3",
                       "o_com3"]
            zT = [pc.tile([64, NL], dt.float32, name=f"zT_{k}")
                  for k in range(4)]
            z_nm_store = []
            for m in range(3):
                sl = slice(MB * m, MB * (m + 1))
                o_nm = pr.tile([MB, 384], dt.float32, tag="onm", bufs=3,
                               name=f"onm_{m}")
                for p in range(3):
                    pt = ps.tile([MB, 128], dt.float32, tag="hps",
                                 name=f"pho_{m}_{p}")
                    nc.tensor.transpose(pt[:], oT[p][:, sl], ident_f[:])
                    nc.vector.tensor_copy(o_nm[:, 128 * p:128 * (p + 1)],
                                          pt[:])

                # batched row-normalize of all 6 blocks
                sq = pr.tile([MB, 384], dt.float32, tag="sq", bufs=3,
                             name=f"sq_{m}")
                nc.vector.tensor_mul(sq[:], o_nm[:], o_nm[:])
                nrm6 = pr.tile([MB, 6], dt.float32, tag="nrm6", bufs=3,
                               name=f"nrm6_{m}")
                nc.vector.reduce_sum(nrm6[:],
                                     sq[:].rearrange("p (s f) -> p s f", f=64),
                                     axis=AX.X)
                nc.vector.tensor_scalar_max(nrm6[:], nrm6[:], eps_t[0:MB, :])
                nc.scalar.activation(nrm6[:], nrm6[:], AF.Sqrt)
                nc.vector.reciprocal(nrm6[:], nrm6[:])

                en = {}
                for k, nm in enumerate(blk_out):
                    e = pc.tile([MB, 64], dt.float32, name=f"en_{m}_{nm}")
                    nc.vector.tensor_scalar_mul(
                        e[:], o_nm[:, 64 * k:64 * (k + 1)], nrm6[:, k:k + 1])
                    nc.sync.dma_start(emb_d[nm][sl, :], e[:])
                    en[nm] = e

                xcom = pc.tile([MB, 64], dt.float32, name=f"xcom_{m}")
                nc.vector.tensor_add(xcom[:], en["o_com1"][:], en["o_com2"][:])
                nc.vector.tensor_add(xcom[:], xcom[:], en["o_com3"][:])
                nc.vector.tensor_scalar_mul(xcom[:], xcom[:], third_t[0:MB, :])

                z_nm = {0: en["o_emb1"], 1: en["o_emb2"], 2: en["o_emb3"],
                        3: xcom}
                z_nm_store.append(z_nm)
                for k in range(4):
                    ptz = ps.tile([64, MB], dt.float32, tag="hps",
                                  name=f"ptz_{m}_{k}")
                    nc.tensor.transpose(ptz[:], z_nm[k][:],
                                        ident_f[0:MB, 0:MB])
                    nc.vector.tensor_copy(zT[k][:, sl], ptz[:])

            # attention scores w_k: [1, 375] each
            wsb = []
            for k in range(4):
                pa = ps.tile([2, NL], dt.float32, tag="hps", name=f"pa_{k}")
                nc.tensor.matmul(pa[:], attw1[:], zT[k][:], start=True,
                                 stop=True)
                a1 = pr.tile([2, NL], dt.float32, tag="a1", bufs=4,
                             name=f"a1_{k}")
                nc.scalar.activation(a1[:], pa[:], AF.Tanh, bias=attb1[:])
                pk = ps.tile([1, NL], dt.float32, tag="hps", name=f"pk_{k}")
                nc.tensor.matmul(pk[:], attw2[:], a1[:], start=True, stop=True)
                w = pc.tile([1, NL], dt.float32, name=f"wsb_{k}")
                nc.vector.tensor_copy(w[:], pk[:])
                wsb.append(w)

            # softmax over the 4 scores (all partition-0 rows)
            mx = pc.tile([1, NL], dt.float32, name="mx")
            nc.vector.tensor_max(mx[:], wsb[0][:], wsb[1][:])
            nc.vector.tensor_max(mx[:], mx[:], wsb[2][:])
            nc.vector.tensor_max(mx[:], mx[:], wsb[3][:])
            es = []
            for k in range(4):
                e = pc.tile([1, NL], dt.float32, name=f"es_{k}")
                nc.vector.tensor_sub(e[:], wsb[k][:], mx[:])
                nc.scalar.activation(e[:], e[:], AF.Exp)
                es.append(e)
            ssum4 = pc.tile([1, NL], dt.float32, name="ssum4")
            nc.vector.tensor_add(ssum4[:], es[0][:], es[1][:])
            nc.vector.tensor_add(ssum4[:], ssum4[:], es[2][:])
            nc.vector.tensor_add(ssum4[:], ssum4[:], es[3][:])
            rcp4 = pc.tile([1, NL], dt.float32, name="rcp4")
            nc.vector.reciprocal(rcp4[:], ssum4[:])
            betas = []
            for k in range(4):
                b = pc.tile([1, NL], dt.float32, name=f"beta_{k}")
                nc.vector.tensor_mul(b[:], es[k][:], rcp4[:])
                betas.append(b)

            # beta -> node-major + emb = sum beta_k * z_k, logits, softmax
            embT = pc.tile([64, NL], dt.float32, name="embT")
            for m in range(3):
                sl = slice(MB * m, MB * (m + 1))
                beta_nm = pr.tile([MB, 4], dt.float32, tag="betanm", bufs=3,
                                  name=f"betanm_{m}")
                for k in range(4):
                    ptb = ps.tile([MB, 1], dt.float32, tag="hps",
                                  name=f"ptb_{m}_{k}")
                    nc.tensor.transpose(ptb[:], betas[k][:, sl],
                                        ident_f[0:1, 0:1])
                    nc.vector.tensor_copy(beta_nm[:, k:k + 1], ptb[:])
                nc.sync.dma_start(o_beta_d[sl, :], beta_nm[:])

                z_nm = z_nm_store[m]
                emb_nm = pr.tile([MB, 64], dt.float32, tag="embnm", bufs=2,
                                 name=f"embnm_{m}")
                tmp = pr.tile([MB, 64], dt.float32, tag="tmpnm", bufs=2,
                              name=f"tmpnm_{m}")
                nc.vector.tensor_scalar_mul(emb_nm[:], z_nm[0][:],
                                            beta_nm[:, 0:1])
                for k in range(1, 4):
                    nc.vector.tensor_scalar_mul(tmp[:], z_nm[k][:],
                                                beta_nm[:, k:k + 1])
                    nc.vector.tensor_add(emb_nm[:], emb_nm[:], tmp[:])

                pte = ps.tile([64, MB], dt.float32, tag="hps",
                              name=f"pte_{m}")
                nc.tensor.transpose(pte[:], emb_nm[:], ident_f[0:MB, 0:MB])
                nc.vector.tensor_copy(embT[:, sl], pte[:])

            pl = ps.tile([NCLASS, NL], dt.float32, tag="hps", name="pl")
            nc.tensor.matmul(pl[:], mlpw[:], embT[:], start=True, stop=True)
            lg = pc.tile([NCLASS, NL], dt.float32, name="lg")
            nc.vector.tensor_scalar_add(lg[:], pl[:], mlpb[:])
            for m in range(3):
                sl = slice(MB * m, MB * (m + 1))
                ptl = ps.tile([MB, NCLASS], dt.float32, tag="hps",
                              name=f"ptl_{m}")
                nc.tensor.transpose(ptl[:], lg[:, sl], ident_f[0:2, 0:2])
                lgn = pr.tile([MB, NCLASS], dt.float32, tag="lgn", bufs=2,
                              name=f"lgn_{m}")
                nc.vector.tensor_copy(lgn[:], ptl[:])
                lmx = pr.tile([MB, 1], dt.float32, tag="lmx", bufs=2,
                              name=f"lmx_{m}")
                nc.vector.reduce_max(lmx[:], lgn[:], axis=AX.X)
                nc.vector.tensor_scalar_sub(lgn[:], lgn[:], lmx[:])
                nc.scalar.activation(lgn[:], lgn[:], AF.Exp)
                lsm = pr.tile([MB, 1], dt.float32, tag="lsm", bufs=2,
                              name=f"lsm_{m}")
                nc.vector.reduce_sum(lsm[:], lgn[:], axis=AX.X)
                lrc = pr.tile([MB, 1], dt.float32, tag="lrc", bufs=2,
                              name=f"lrc_{m}")
                nc.vector.reciprocal(lrc[:], lsm[:])
                nc.vector.tensor_scalar_mul(lgn[:], lgn[:], lrc[:])
                nc.sync.dma_start(o_output_d[sl, :], lgn[:])
            ps_head.__exit__(None, None, None)

    nc.compile()
    return nc


_NC_CACHE = None


def _get_nc():
    global _NC_CACHE
    if _NC_CACHE is None:
        _NC_CACHE = build()
    return _NC_CACHE


def kernel(x, sadj, fadj, fadj2, sgcn1, sgcn2, sgcn3, cgcn,
           att_w1, att_b1, att_w2, mlp_w, mlp_b):
    x = np.asarray(x, np.float32)
    pairs = [(sgcn1, cgcn), (sgcn2, cgcn), (sgcn3, cgcn)]
    adjs = [sadj, fadj, fadj2]

    wblobs = [_pack_weights_img(PA, PB) for PA, PB in pairs]
    biases = [_pack_pair_bias(PA, PB) for PA, PB in pairs]
    shared = {
        "attw1": np.asarray(att_w1, np.float32).reshape(OUT, 2),
        "attb1": np.asarray(att_b1, np.float32).reshape(2, 1),
        "attw2": np.asarray(att_w2, np.float32).reshape(2, 1),
        "mlpw": np.asarray(mlp_w, np.float32).reshape(OUT, NCLASS),
        "mlpb": np.asarray(mlp_b, np.float32).reshape(NCLASS, 1),
    }

    in_maps = []
    for c in range(NCORES):
        rows = slice(NL * c, NL * (c + 1))
        m = {
            "xT": np.ascontiguousarray(x[rows].T).astype(bf16),
        }
        for p in range(3):
            m[f"adjT{p}"] = _prep_adjT_img(adjs[p], rows)
            m[f"wblob{p}"] = wblobs[p]
            m[f"bias{p}"] = biases[p]
        m.update(shared)
        in_maps.append(m)

    nc = _get_nc()
    trace = bool(int(os.environ.get("KERNEL_TRACE", "0")))
    res = run_bass_kernel_spmd(nc, in_maps, core_ids=list(range(NCORES)),
                               trace=trace)
    if trace:
        kernel.last_exec_time_ns = res.exec_time_ns
        kernel.last_results = res

    def cat(name):
        return np.concatenate([res.results[c][name] for c in range(NCORES)],
                              axis=0)

    output = cat("o_output")
    beta = cat("o_beta").reshape(N, 4, 1)
    emb1 = cat("o_emb1")
    com1 = cat("o_com1")
    com2 = cat("o_com2")
    com3 = cat("o_com3")
    emb2 = cat("o_emb2")
    emb3 = cat("o_emb3")
    return (output, beta, emb1, com1, com2, com3, emb2, emb3)


# revision 11
# speedup vs baseline: 1.0908x; 1.0865x over previous
"""MAMFGCN Trainium2 kernel: 6 snowball GCNs + attention fusion on 8 NeuronCores.

Strategy:
- Row-shard the node dim N=3000 across 8 cores (375 nodes each).
- The 6 snowballs (emb1,com1 | emb2,com2 | emb3,com3) are grouped into 3
  pairs, one per adjacency (sadj, fadj, fadj2), and run in lockstep.
- Everything on-chip lives feature-major ("transposed"): featsT tiles are
  [feat, node]. Heavy matmuls put the 375-node dim as the moving free dim
  (>=256 -> full PE rate).
- Per layer: XW^T = W^T @ featsT (per pair, both snowballs packed in 128
  PSUM partitions) -> PE-transpose to node-major -> ONE AllGather of the
  batched [3000, 384] XW (bf16) -> h^T = XWfull^T-stationary @ adjT-moving
  accumulated over 24 node k-tiles -> tanh(+bias) lands h^T directly in the
  next layer's featsT layout.
- Emission order: all of layer li+1's XW^T matmuls except the newest-h term
  are emitted right after AG(li) so the PE has work during the gather.
- DMAs are batched with 3D access patterns to keep the sync engine cheap.
- Adjacencies are host-transposed/padded once and stay SBUF-resident (bf16).
- Final attention/softmax head is computed on-chip in fp32.
"""
import os
import sys

sys.path.insert(0, "/opt/trn_rl_repo")
import numpy as np
import ml_dtypes

import concourse.bass as bass
import concourse.mybir as mybir
import concourse.tile as tile
from concourse import bacc
from concourse.bass_utils import run_bass_kernel_spmd
from concourse.masks import make_identity

dt = mybir.dt
AF = mybir.ActivationFunctionType
AX = mybir.AxisListType

N, NFEAT, NHID, NLAYERS, OUT, NCLASS = 3000, 256, 64, 9, 64, 2
NCORES = 8
NL = N // NCORES          # 375 local nodes
NPAD = 384                # per-rank padded rows for the AllGather
KN = NPAD * NCORES        # 3072 padded global nodes
KT = KN // 128            # 24 node k-tiles
KC = 4                    # xwf DMA chunks per layer (6 k-tiles each)
MB = 125                  # node-major m-chunk size (3 chunks of 125)
XWW = 6 * NHID            # 384 = width of the batched XW
bf16 = ml_dtypes.bfloat16

# tile-offset of each layer's weight tiles inside a pair's blob
_WOFF = []
_off = 0
for _i in range(NLAYERS):
    _WOFF.append(_off)
    _off += 2 + _i
_WOFF.append(_off)          # output layer: 2 + NLAYERS tiles
_WTILES = _off + 2 + NLAYERS  # 65 tiles per pair


def _pack_pair_weights(PA, PB):
    """[65*128, 128] f32 blob of PE-ready lhsT tiles for one pair."""
    tiles = []
    for i in range(NLAYERS + 1):
        WA = np.asarray(PA["Ws"][i] if i < NLAYERS else PA["Wo"], np.float32)
        WB = np.asarray(PB["Ws"][i] if i < NLAYERS else PB["Wo"], np.float32)
        nh = NLAYERS if i == NLAYERS else i
        for t in range(2):  # x-part: both snowballs packed on M
            tl = np.zeros((128, 128), np.float32)
            tl[:, 0:64] = WA[128 * t:128 * (t + 1), :]
            tl[:, 64:128] = WB[128 * t:128 * (t + 1), :]
            tiles.append(tl)
        for j in range(nh):  # h-part: block-diagonal (A top-left, B bottom-right)
            tl = np.zeros((128, 128), np.float32)
            tl[0:64, 0:64] = WA[256 + 64 * j:320 + 64 * j, :]
            tl[64:128, 64:128] = WB[256 + 64 * j:320 + 64 * j, :]
            tiles.append(tl)
    return np.concatenate(tiles, axis=0)


def _pack_pair_bias(PA, PB):
    """[128, 10] f32: col i = [b_A_i ; b_B_i], col 9 = output bias."""
    out = np.zeros((128, NLAYERS + 1), np.float32)
    for i in range(NLAYERS):
        out[0:64, i] = np.asarray(PA["bs"][i], np.float32)
        out[64:128, i] = np.asarray(PB["bs"][i], np.float32)
    out[0:64, NLAYERS] = np.asarray(PA["bo"], np.float32)
    out[64:128, NLAYERS] = np.asarray(PB["bo"], np.float32)
    return out


def _prep_adjT_img(adj, rows):
    """SBUF image [128, KT*NL]: partition p, col kt*NL+n = adjT[128*kt+p, n]."""
    a = _prep_adjT(adj, rows)                       # [KN, NL]
    return np.ascontiguousarray(
        a.reshape(KT, 128, NL).transpose(1, 0, 2).reshape(128, KT * NL))


def _pack_weights_img(PA, PB):
    """SBUF image [128, 65*128] of the pair's lhsT tiles."""
    b = _pack_pair_weights(PA, PB).astype(bf16)     # [65*128, 128]
    return np.ascontiguousarray(
        b.reshape(_WTILES, 128, 128).transpose(1, 0, 2).reshape(128, _WTILES * 128))


def _prep_adjT(adj, rows, dtype=bf16):
    """[KN, NL]: row 384*r+j = adj[rows, 375*r+j], pad rows zero."""
    out = np.zeros((KN, NL), dtype)
    a = np.asarray(adj, np.float32)
    for r in range(NCORES):
        blk = a[rows, NL * r:NL * (r + 1)].T.astype(dtype)  # [375, 375]
        out[NPAD * r:NPAD * r + NL, :] = blk
    return out


def build():
    nc = bacc.Bacc("TRN2", target_bir_lowering=False, debug=False,
                   num_devices=NCORES)

    xT_d = nc.dram_tensor("xT", [NFEAT, NL], dt.bfloat16, kind="ExternalInput")
    adjT_d = [nc.dram_tensor(f"adjT{p}", [128, KT * NL], dt.bfloat16,
                             kind="ExternalInput") for p in range(3)]
    wb_d = [nc.dram_tensor(f"wblob{p}", [128, _WTILES * 128], dt.bfloat16,
                           kind="ExternalInput") for p in range(3)]
    bias_d = [nc.dram_tensor(f"bias{p}", [128, NLAYERS + 1], dt.float32,
                             kind="ExternalInput") for p in range(3)]
    attw1_d = nc.dram_tensor("attw1", [OUT, 2], dt.float32, kind="ExternalInput")
    attb1_d = nc.dram_tensor("attb1", [2, 1], dt.float32, kind="ExternalInput")
    attw2_d = nc.dram_tensor("attw2", [2, 1], dt.float32, kind="ExternalInput")
    mlpw_d = nc.dram_tensor("mlpw", [OUT, NCLASS], dt.float32, kind="ExternalInput")
    mlpb_d = nc.dram_tensor("mlpb", [NCLASS, 1], dt.float32, kind="ExternalInput")

    out_names = ["o_output", "o_beta", "o_emb1", "o_com1", "o_com2",
                 "o_com3", "o_emb2", "o_emb3"]
    o_output_d = nc.dram_tensor("o_output", [NL, NCLASS], dt.float32,
                                kind="ExternalOutput")
    o_beta_d = nc.dram_tensor("o_beta", [NL, 4], dt.float32,
                              kind="ExternalOutput")
    emb_d = {}
    for nm in out_names[2:]:
        emb_d[nm] = nc.dram_tensor(nm, [NL, OUT], dt.float32,
                                   kind="ExternalOutput")

    with tile.TileContext(nc) as tc:
        with (
            tc.tile_pool(name="const", bufs=1) as pc,
            tc.tile_pool(name="wts", bufs=4) as pw,
            tc.tile_pool(name="xwf", bufs=6) as pxwf,
            tc.tile_pool(name="rot", bufs=2) as pr,
            tc.tile_pool(name="dram", bufs=1, space="DRAM") as pd,
        ):
            # ---- persistent/const tiles ----
            ident_b = pc.tile([128, 128], dt.bfloat16, name="ident_b")
            make_identity(nc, ident_b[:])
            ident_f = pc.tile([128, 128], dt.float32, name="ident_f")
            make_identity(nc, ident_f[:])
            zeros_b = pc.tile([128, XWW], dt.bfloat16, name="zeros_b")
            nc.gpsimd.memset(zeros_b[:], 0.0)

            xT = []
            for t in range(2):
                xt = pc.tile([128, NL], dt.bfloat16, name=f"xT_{t}")
                nc.sync.dma_start(xt[:], xT_d[128 * t:128 * (t + 1), :])
                xT.append(xt)

            bias = []
            for p in range(3):
                b = pc.tile([128, NLAYERS + 1], dt.float32, name=f"bias_{p}")
                nc.sync.dma_start(b[:], bias_d[p][:])
                bias.append(b)

            # adjacencies: SBUF image layout, 2 contiguous DMAs per matrix
            adjT = []
            for p in range(3):
                a = pc.tile([128, KT * NL], dt.bfloat16, name=f"adjT_{p}")
                half = KT * NL // 2
                nc.sync.dma_start(a[:, 0:half], adjT_d[p][:, 0:half])
                nc.sync.dma_start(a[:, half:], adjT_d[p][:, half:])
                adjT.append(a)

            attw1 = pc.tile([OUT, 2], dt.float32, name="attw1")
            nc.sync.dma_start(attw1[:], attw1_d[:])
            attb1 = pc.tile([2, 1], dt.float32, name="attb1")
            nc.sync.dma_start(attb1[:], attb1_d[:])
            attw2 = pc.tile([2, 1], dt.float32, name="attw2")
            nc.sync.dma_start(attw2[:], attw2_d[:])
            mlpw = pc.tile([OUT, NCLASS], dt.float32, name="mlpw")
            nc.sync.dma_start(mlpw[:], mlpw_d[:])
            mlpb = pc.tile([NCLASS, 1], dt.float32, name="mlpb")
            nc.sync.dma_start(mlpb[:], mlpb_d[:])

            eps_t = pc.tile([128, 1], dt.float32, name="eps_t")
            nc.gpsimd.memset(eps_t[:], 1e-24)
            third_t = pc.tile([128, 1], dt.float32, name="third_t")
            nc.gpsimd.memset(third_t[:], 1.0 / 3.0)

            hT = [[None] * NLAYERS for _ in range(3)]  # persistent h tiles
            oT = [None] * 3

            # ---- layer loop: two pipelined groups ----
            # Group A = pairs {0,1}, group B = pair {2}, with B shifted half
            # a step so the collective stream and the PE stay concurrently
            # busy: adj_B(i-1) and adj_A(i) execute under AG_A(i) / AG_B(i).
            ps_main = tc.tile_pool(name="ps_main", bufs=2, space="PSUM")
            ps = ps_main.__enter__()

            def load_w(li, p, t0, ntl, nm):
                """One contiguous DMA for weight tiles t0..t0+ntl."""
                w = pw.tile([128, ntl, 128], dt.bfloat16, tag="wt",
                            padded_shape=[128, NLAYERS + 2, 128], bufs=4,
                            name=nm)
                c0 = (_WOFF[li] + t0) * 128
                nc.sync.dma_start(
                    w[:], wb_d[p][:, c0:c0 + ntl * 128]
                    .rearrange("p (t m) -> p t m", m=128))
                return w

            GA, GB = [0, 1], [2]
            GW = {0: 2 * 128, 1: 128}          # AG width per group
            xwf_cur = {}                        # group -> xwf chunk tiles
            pxw_cur = [None] * 3

            def partials(li):
                """XW^T(li) minus the newest-h term (ready during AGs)."""
                for p in range(3):
                    pxw = ps.tile([128, NL], dt.float32, tag="pxw", bufs=4,
                                  name=f"pxw_{li}_{p}")
                    w = load_w(li, p, 0, 2 + max(0, li - 1), f"w_{li}_{p}")
                    nc.tensor.matmul(pxw[:], w[:, 0, :], xT[0][:],
                                     start=True, stop=(li == 0))
                    nc.tensor.matmul(pxw[:], w[:, 1, :], xT[1][:],
                                     start=False, stop=(li == 0))
                    for j in range(li - 1):
                        nc.tensor.matmul(pxw[:], w[:, 2 + j, :], hT[p][j][:],
                                         start=False, stop=False)
                    pxw_cur[p] = pxw

            def finish(g, li, cc_in, cc_out):
                gprs = GA if g == 0 else GB
                nc.sync.dma_start(cc_in[NL:NPAD, :],
                                  zeros_b[0:NPAD - NL, 0:GW[g]])
                ccs = [pr.tile([MB, GW[g]], dt.bfloat16, tag=f"ccs{g}",
                               bufs=4, name=f"ccs_{g}_{li}_{m}")
                       for m in range(3)]
                for pi, p in enumerate(gprs):
                    pxw = pxw_cur[p]
                    if li >= 1:
                        wl = load_w(li, p, 2 + (li - 1), 1, f"wl_{li}_{p}")
                        nc.tensor.matmul(pxw[:], wl[:, 0, :],
                                         hT[p][li - 1][:],
                                         start=False, stop=True)
                    xwtT = pr.tile([128, NL], dt.bfloat16, tag="xwtT",
                                   bufs=3, name=f"xwtT_{li}_{p}")
                    nc.scalar.activation(xwtT[:], pxw[:], AF.Copy)
                    for m in range(3):
                        ptr = ps.tile([MB, 128], dt.bfloat16, tag="ptr",
                                      bufs=2, name=f"ptr_{li}_{p}_{m}")
                        nc.tensor.transpose(
                            ptr[:], xwtT[:, MB * m:MB * (m + 1)], ident_b[:])
                        nc.vector.tensor_copy(
                            ccs[m][:, 128 * pi:128 * (pi + 1)], ptr[:])
                for m in range(3):
                    nc.sync.dma_start(cc_in[MB * m:MB * (m + 1), :], ccs[m][:])
                nc.gpsimd.collective_compute(
                    "AllGather", mybir.AluOpType.bypass,
                    replica_groups=[list(range(NCORES))],
                    ins=[cc_in.opt()], outs=[cc_out.opt()],
                )
                xwf = []
                for kc in range(KC):
                    xf = pxwf.tile([128, KT // KC, GW[g]], dt.bfloat16,
                                   tag=f"xwf{g}", bufs=6,
                                   name=f"xwf_{g}_{li}_{kc}")
                    r0 = kc * (KN // KC)
                    nc.sync.dma_start(
                        xf[:], cc_out[r0:r0 + KN // KC, :]
                        .rearrange("(t p) f -> p t f", p=128))
                    xwf.append(xf)
                xwf_cur[g] = xwf

            def adj_phase(g, li):
                gprs = GA if g == 0 else GB
                xwf = xwf_cur[g]
                for pi, p in enumerate(gprs):
                    ph = ps.tile([128, NL], dt.float32, tag="ph", bufs=2,
                                 name=f"ph_{li}_{p}")
                    for kt in range(KT):
                        nc.tensor.matmul(
                            ph[:],
                            xwf[kt // 6][:, kt % 6, 128 * pi:128 * (pi + 1)],
                            adjT[p][:, kt * NL:(kt + 1) * NL],
                            start=(kt == 0), stop=(kt == KT - 1))
                    if li < NLAYERS:
                        h = pc.tile([128, NL], dt.bfloat16,
                                    name=f"hT_{p}_{li}")
                        nc.scalar.activation(h[:], ph[:], AF.Tanh,
                                             bias=bias[p][:, li:li + 1])
                        hT[p][li] = h
                    else:
                        o = pc.tile([128, NL], dt.float32, name=f"oT_{p}")
                        nc.vector.tensor_scalar_add(o[:], ph[:],
                                                    bias[p][:, li:li + 1])
                        oT[p] = o

            # per-layer collective buffers
            ccb = {}
            for li in range(NLAYERS + 1):
                for g in (0, 1):
                    ci = pd.tile([NPAD, GW[g]], dt.bfloat16,
                                 name=f"cc_in_{g}_{li}")
                    co = pd.tile([KN, GW[g]], dt.bfloat16,
                                 name=f"cc_out_{g}_{li}", addr_space="Shared")
                    ccb[(g, li)] = (ci, co)

            partials(0)
            for li in range(NLAYERS + 1):
                finish(0, li, *ccb[(0, li)])          # -> AG_A(li)
                if li >= 1:
                    adj_phase(1, li - 1)              # under AG_A(li)
                finish(1, li, *ccb[(1, li)])          # -> AG_B(li)
                if li < NLAYERS:
                    partials(li + 1)                  # under AG_A/AG_B
                adj_phase(0, li)                      # under AG_B(li)
            adj_phase(1, NLAYERS)                     # epilogue: pair 2 out

            ps_main.__exit__(None, None, None)
            ps_head = tc.tile_pool(name="ps_head", bufs=8, space="PSUM")
            ps = ps_head.__enter__()

            # ---- head (fp32) ----
            # snowball order inside pair tiles: cols [0:64]=A, [64:128]=B
            # block order across [125, 384] node-major tiles:
            #   0=emb1 1=com1 2=emb2 3=com2 4=emb3 5=com3
            blk_out = ["o_emb1", "o_com1", "o_emb2", "o_com2", "o_emb3",
                       "o_com3"]
            zT = [pc.tile([64, NL], dt.float32, name=f"zT_{k}")
                  for k in range(4)]
            z_nm_store = []
            for m in range(3):
                sl = slice(MB * m, MB * (m + 1))
                o_nm = pr.tile([MB, 384], dt.float32, tag="onm", bufs=3,
                               name=f"onm_{m}")
                for p in range(3):
                    pt = ps.tile([MB, 128], dt.float32, tag="hps",
                                 name=f"pho_{m}_{p}")
                    nc.tensor.transpose(pt[:], oT[p][:, sl], ident_f[:])
                    nc.vector.tensor_copy(o_nm[:, 128 * p:128 * (p + 1)],
                                          pt[:])

                # batched row-normalize of all 6 blocks
                sq = pr.tile([MB, 384], dt.float32, tag="sq", bufs=3,
                             name=f"sq_{m}")
                nc.vector.tensor_mul(sq[:], o_nm[:], o_nm[:])
                nrm6 = pr.tile([MB, 6], dt.float32, tag="nrm6", bufs=3,
                               name=f"nrm6_{m}")
                nc.vector.reduce_sum(nrm6[:],
                                     sq[:].rearrange("p (s f) -> p s f", f=64),
                                     axis=AX.X)
                nc.vector.tensor_scalar_max(nrm6[:], nrm6[:], eps_t[0:MB, :])
                nc.scalar.activation(nrm6[:], nrm6[:], AF.Sqrt)
                nc.vector.reciprocal(nrm6[:], nrm6[:])

                en = {}
                for k, nm in enumerate(blk_out):
                    e = pc.tile([MB, 64], dt.float32, name=f"en_{m}_{nm}")
                    nc.vector.tensor_scalar_mul(
                        e[:], o_nm[:, 64 * k:64 * (k + 1)], nrm6[:, k:k + 1])
                    nc.sync.dma_start(emb_d[nm][sl, :], e[:])
                    en[nm] = e

                xcom = pc.tile([MB, 64], dt.float32, name=f"xcom_{m}")
                nc.vector.tensor_add(xcom[:], en["o_com1"][:], en["o_com2"][:])
                nc.vector.tensor_add(xcom[:], xcom[:], en["o_com3"][:])
                nc.vector.tensor_scalar_mul(xcom[:], xcom[:], third_t[0:MB, :])

                z_nm = {0: en["o_emb1"], 1: en["o_emb2"], 2: en["o_emb3"],
                        3: xcom}
                z_nm_store.append(z_nm)
                for k in range(4):
                    ptz = ps.tile([64, MB], dt.float32, tag="hps",
                                  name=f"ptz_{m}_{k}")
                    nc.tensor.transpose(ptz[:], z_nm[k][:],
                                        ident_f[0:MB, 0:MB])
                    nc.vector.tensor_copy(zT[k][:, sl], ptz[:])

            # attention scores w_k: [1, 375] each
            wsb = []
            for k in range(4):
                pa = ps.tile([2, NL], dt.float32, tag="hps", name=f"pa_{k}")
                nc.tensor.matmul(pa[:], attw1[:], zT[k][:], start=True,
                                 stop=True)
                a1 = pr.tile([2, NL], dt.float32, tag="a1", bufs=4,
                             name=f"a1_{k}")
                nc.scalar.activation(a1[:], pa[:], AF.Tanh, bias=attb1[:])
                pk = ps.tile([1, NL], dt.float32, tag="hps", name=f"pk_{k}")
                nc.tensor.matmul(pk[:], attw2[:], a1[:], start=True, stop=True)
                w = pc.tile([1, NL], dt.float32, name=f"wsb_{k}")
                nc.vector.tensor_copy(w[:], pk[:])
                wsb.append(w)

            # softmax over the 4 scores (all partition-0 rows)
            mx = pc.tile([1, NL], dt.float32, name="mx")
            nc.vector.tensor_max(mx[:], wsb[0][:], wsb[1][:])
            nc.vector.tensor_max(mx[:], mx[:], wsb[2][:])
            nc.vector.tensor_max(mx[:], mx[:], wsb[3][:])
            es = []
            for k in range(4):
                e = pc.tile([1, NL], dt.float32, name=f"es_{k}")
                nc.vector.tensor_sub(e[:], wsb[k][:], mx[:])
                nc.scalar.activation(e[:], e[:], AF.Exp)
                es.append(e)
            ssum4 = pc.tile([1, NL], dt.float32, name="ssum4")
            nc.vector.tensor_add(ssum4[:], es[0][:], es[1][:])
            nc.vector.tensor_add(ssum4[:], ssum4[:], es[2][:])
            nc.vector.tensor_add(ssum4[:], ssum4[:], es[3][:])
            rcp4 = pc.tile([1, NL], dt.float32, name="rcp4")
            nc.vector.reciprocal(rcp4[:], ssum4[:])
            betas = []
            for k in range(4):
                b = pc.tile([1, NL], dt.float32, name=f"beta_{k}")
                nc.vector.tensor_mul(b[:], es[k][:], rcp4[:])
                betas.append(b)

            # beta -> node-major + emb = sum beta_k * z_k, logits, softmax
            embT = pc.tile([64, NL], dt.float32, name="embT")
            for m in range(3):
                sl = slice(MB * m, MB * (m + 1))
                beta_nm = pr.tile([MB, 4], dt.float32, tag="betanm", bufs=3,
                                  name=f"betanm_{m}")
                for k in range(4):
                    ptb = ps.tile([MB, 1], dt.float32, tag="hps",
                                  name=f"ptb_{m}_{k}")
                    nc.tensor.transpose(ptb[:], betas[k][:, sl],
                                        ident_f[0:1, 0:1])
                    nc.vector.tensor_copy(beta_nm[:, k:k + 1], ptb[:])
                nc.sync.dma_start(o_beta_d[sl, :], beta_nm[:])

                z_nm = z_nm_store[m]
                emb_nm = pr.tile([MB, 64], dt.float32, tag="embnm", bufs=2,
                                 name=f"embnm_{m}")
                tmp = pr.tile([MB, 64], dt.float32, tag="tmpnm", bufs=2,
                              name=f"tmpnm_{m}")
                nc.vector.tensor_scalar_mul(emb_nm[:], z_nm[0][:],
                                            beta_nm[:, 0:1])
                for k in range(1, 4):
                    nc.vector.tensor_scalar_mul(tmp[:], z_nm[k][:],
                                                beta_nm[:, k:k + 1])
                    nc.vector.tensor_add(emb_nm[:], emb_nm[:], tmp[:])

                pte = ps.tile([64, MB], dt.float32, tag="hps",
                              name=f"pte_{m}")
                nc.tensor.transpose(pte[:], emb_nm[:], ident_f[0:MB, 0:MB])
                nc.vector.tensor_copy(embT[:, sl], pte[:])

            pl = ps.tile([NCLASS, NL], dt.float32, tag="hps", name="pl")
            nc.tensor.matmul(pl[:], mlpw[:], embT[:], start=True, stop=True)
            lg = pc.tile([NCLASS, NL], dt.float32, name="lg")
            nc.vector.tensor_scalar_add(lg[:], pl[:], mlpb[:])
            for m in range(3):
                sl = slice(MB * m, MB * (m + 1))
                ptl = ps.tile([MB, NCLASS], dt.float32, tag="hps",
                              name=f"ptl_{m}")
                nc.tensor.transpose(ptl[:], lg[:, sl], ident_f[0:2, 0:2])
                lgn = pr.tile([MB, NCLASS], dt.float32, tag="lgn", bufs=2,
                              name=f"lgn_{m}")
                nc.vector.tensor_copy(lgn[:], ptl[:])
                lmx = pr.tile([MB, 1], dt.float32, tag="lmx", bufs=2,
                              name=f"lmx_{m}")
                nc.vector.reduce_max(lmx[:], lgn[:], axis=AX.X)
                nc.vector.tensor_scalar_sub(lgn[:], lgn[:], lmx[:])
                nc.scalar.activation(lgn[:], lgn[:], AF.Exp)
                lsm = pr.tile([MB, 1], dt.float32, tag="lsm", bufs=2,
                              name=f"lsm_{m}")
                nc.vector.reduce_sum(lsm[:], lgn[:], axis=AX.X)
                lrc = pr.tile([MB, 1], dt.float32, tag="lrc", bufs=2,
                              name=f"lrc_{m}")
                nc.vector.reciprocal(lrc[:], lsm[:])
                nc.vector.tensor_scalar_mul(lgn[:], lgn[:], lrc[:])
                nc.sync.dma_start(o_output_d[sl, :], lgn[:])
            ps_head.__exit__(None, None, None)

    nc.compile()
    return nc


_NC_CACHE = None


def _get_nc():
    global _NC_CACHE
    if _NC_CACHE is None:
        _NC_CACHE = build()
    return _NC_CACHE


def kernel(x, sadj, fadj, fadj2, sgcn1, sgcn2, sgcn3, cgcn,
           att_w1, att_b1, att_w2, mlp_w, mlp_b):
    x = np.asarray(x, np.float32)
    pairs = [(sgcn1, cgcn), (sgcn2, cgcn), (sgcn3, cgcn)]
    adjs = [sadj, fadj, fadj2]

    wblobs = [_pack_weights_img(PA, PB) for PA, PB in pairs]
    biases = [_pack_pair_bias(PA, PB) for PA, PB in pairs]
    shared = {
        "attw1": np.asarray(att_w1, np.float32).reshape(OUT, 2),
        "attb1": np.asarray(att_b1, np.float32).reshape(2, 1),
        "attw2": np.asarray(att_w2, np.float32).reshape(2, 1),
        "mlpw": np.asarray(mlp_w, np.float32).reshape(OUT, NCLASS),
        "mlpb": np.asarray(mlp_b, np.float32).reshape(NCLASS, 1),
    }

    in_maps = []
    for c in range(NCORES):
        rows = slice(NL * c, NL * (c + 1))
        m = {
            "xT": np.ascontiguousarray(x[rows].T).astype(bf16),
        }
        for p in range(3):
            m[f"adjT{p}"] = _prep_adjT_img(adjs[p], rows)
            m[f"wblob{p}"] = wblobs[p]
            m[f"bias{p}"] = biases[p]
        m.update(shared)
        in_maps.append(m)

    nc = _get_nc()
    trace = bool(int(os.environ.get("KERNEL_TRACE", "0")))
    res = run_bass_kernel_spmd(nc, in_maps, core_ids=list(range(NCORES)),
                               trace=trace)
    if trace:
        kernel.last_exec_time_ns = res.exec_time_ns
        kernel.last_results = res

    def cat(name):
        return np.concatenate([res.results[c][name] for c in range(NCORES)],
                              axis=0)

    output = cat("o_output")
    beta = cat("o_beta").reshape(N, 4, 1)
    emb1 = cat("o_emb1")
    com1 = cat("o_com1")
    com2 = cat("o_com2")
    com3 = cat("o_com3")
    emb2 = cat("o_emb2")
    emb3 = cat("o_emb3")
    return (output, beta, emb1, com1, com2, com3, emb2, emb3)
